# revision 2
# baseline (speedup 1.0000x reference)
"""Trainium2 Bass kernel for nn_BertClassifier_77309411685 — v4.

Data-parallel over 8 NeuronCores: 256 samples/core; base linear + 12 expert
heads replicated (host-cast bf16).

The masked span-sum is computed BY THE DMA ENGINES: indirect gathers with
compute_op=add accumulate 2-row pair chunks into SBUF; rows beyond each
span are skip-marked via the DGE bounds check so HBM traffic is the exact
4.5-row average.  Accumulation chains are kept one level deep (p2 adds onto
p0's tile, p3 onto p1's; the odd-tail row lands in its own tile) and are
emitted so every WAW wait is already drained when the Q7 reaches it.

Per half: carrier tile holds A=[rows 0-1 (+4-5)], B=[rows 2-3 (+6-7)],
O=[odd tail row].  center = (A_L+A_R)/len + (B_L+B_R)/len + O/len computed
as X=A_L+A_R, X'=X/len + O/len, then (tail) Y=B_L+B_R, center=Y/len + X' —
only the last two ops trail the final gather.  The center transposes, ctx
transposes, base linear (per-(mt,half) accumulators) and expert select run
as in v2; a junk-matmul warmup keeps the PE HAM un-throttled.
"""

import numpy as np
from contextlib import ExitStack

import concourse.bass as bass
import concourse.tile as tile
from concourse import bacc, mybir
from concourse.bass import IndirectOffsetOnAxis
from concourse.bass_utils import run_bass_kernel_spmd

import ml_dtypes

F32 = mybir.dt.float32
BF16 = mybir.dt.bfloat16
I32 = mybir.dt.int32

B, S, H = 2048, 256, 768
INNER, NB_CTX, NB_EXPERTS, NB_LABELS = 256, 2, 12, 3
NCORES = 8
BC = B // NCORES
F3H = (NB_CTX + 1) * H
KC = F3H // 128
NE = NB_EXPERTS * NB_LABELS
HC = H // 128

CTX_IDX = [int(v) for v in np.random.default_rng(seed=0).choice(np.arange(S), size=NB_CTX)]

# f32 const blob columns
C_ID, C_IO36, C_BB, C_WA, C_WB, C_WC, C_NF = 0, 128, 164, 166, 202, 238, 274


def _build():
    nc = bacc.Bacc(
        "TRN2",
        target_bir_lowering=False,
        debug=False,
        enable_asserts=False,
        num_devices=NCORES,
    )
    emb = nc.dram_tensor("emb", [BC * S, H], F32, kind="ExternalInput").ap()
    wbT = nc.dram_tensor("wbT", [F3H, INNER], BF16, kind="ExternalInput").ap()
    cstf = nc.dram_tensor("cstf", [128, C_NF], F32, kind="ExternalInput").ap()
    csti = nc.dram_tensor("csti", [128, 14], I32, kind="ExternalInput").ap()
    out = nc.dram_tensor("out", [BC, NB_LABELS], F32, kind="ExternalOutput").ap()

    emb3d = emb.rearrange("(b s) h -> b s h", s=S)

    with tile.TileContext(nc) as tc, ExitStack() as ctx:
        pool = ctx.enter_context(tc.tile_pool(name="main", bufs=1))
        gpool = ctx.enter_context(tc.tile_pool(name="gp", bufs=2))
        spool = ctx.enter_context(tc.tile_pool(name="small", bufs=2))
        pst = ctx.enter_context(tc.tile_pool(name="pst", bufs=3, space="PSUM"))
        psh = ctx.enter_context(tc.tile_pool(name="psh", bufs=2, space="PSUM"))

        # --- consts ---
        csti_t = pool.tile([128, 14], I32)
        nc.sync.dma_start(csti_t[:], csti[:, :])
        cstf_t = pool.tile([128, C_NF], F32)
        nc.sync.dma_start(cstf_t[:], cstf[:, :])

        id_f32 = cstf_t[:, C_ID:C_ID + 128]
        io36f = cstf_t[:, C_IO36:C_IO36 + NE]
        bb_t = cstf_t[:, C_BB:C_BB + 2]

        # DMA accumulators: one carrier per half, zeroed with a single memset.
        # A = [0:2H) rows {0,1}+{4,5};  B = [2H:4H) rows {2,3}+{6,7};
        # O = [4H:5H) odd tail row.
        cars = []
        for h in range(2):
            car = gpool.tile([128, 5 * H], F32, tag=f"car{h}", bufs=1)
            nc.vector.memset(car[:], 0.0)
            cars.append(car)

        # --- PE warmup (HAM un-throttle) on junk data ---
        for w in range(8):
            junk = pst.tile([128, 128], F32, tag="tpc", name=f"junk{w}")
            nc.tensor.matmul(junk[:], lhsT=id_f32, rhs=cstf_t[:, 0:128],
                             start=True, stop=True)

        # --- accumulate-gathers (SWDGE, CCE-add) ---
        ADD = mybir.AluOpType.add
        BYP = mybir.AluOpType.bypass

        def gop(h, k, sl, op):
            nc.gpsimd.indirect_dma_start(
                out=cars[h][:, sl], out_offset=None, in_=emb,
                in_offset=IndirectOffsetOnAxis(
                    ap=csti_t[:, 5 * h + k:5 * h + k + 1], axis=0),
                bounds_check=BC * S - 1, oob_is_err=False,
                compute_op=op,
            )

        A, Bsl, O = slice(0, 2 * H), slice(2 * H, 4 * H), slice(4 * H, 5 * H)
        for h in range(2):          # layer 1: independent writes
            gop(h, 0, A, BYP)       # rows 0-1 (live len>=2)
            gop(h, 1, Bsl, BYP)     # rows 2-3 (live len>=4)
            gop(h, 4, O, BYP)       # odd tail row (live len odd)
        for h in range(2):          # layer 2: single-depth accumulates
            gop(h, 2, A, ADD)       # rows 4-5 (live len>=6)
            gop(h, 3, Bsl, ADD)     # rows 6-7 (live len==8)

        # --- span lengths -> 1/len ---
        rcps = []
        for h in range(2):
            len_f = spool.tile([128, 1], F32, tag=f"lenf{h}", bufs=1)
            nc.vector.tensor_copy(len_f[:], csti_t[:, 10 + h:11 + h])
            rcp = spool.tile([128, 1], F32, tag=f"rcp{h}", bufs=1)
            nc.vector.reciprocal(rcp[:], len_f[:])
            rcps.append(rcp)

        id_bf = pool.tile([128, 128], BF16)
        nc.vector.tensor_copy(id_bf[:], id_f32)
        wexpA = pool.tile([128, NE], BF16)
        nc.vector.tensor_copy(wexpA[:], cstf_t[:, C_WA:C_WA + NE])
        wexpB = pool.tile([128, NE], BF16)
        nc.vector.tensor_copy(wexpB[:], cstf_t[:, C_WB:C_WB + NE])
        wexpC = pool.tile([1, NE], BF16)
        nc.vector.tensor_copy(wexpC[:], cstf_t[0:1, C_WC:C_WC + NE])
        ones1 = pool.tile([1, 256], BF16)
        nc.vector.memset(ones1[:], 1.0)

        # --- context rows (scalar HWDGE queue) + wbT (sync queue) ---
        ctxs = []
        for h in range(2):
            b0 = h * 128
            ctx0 = gpool.tile([128, H], F32, tag=f"ctx0{h}", bufs=1)
            nc.scalar.dma_start(ctx0[:], emb3d[b0:b0 + 128, CTX_IDX[0], :])
            ctx1 = gpool.tile([128, H], F32, tag=f"ctx1{h}", bufs=1)
            nc.scalar.dma_start(ctx1[:], emb3d[b0:b0 + 128, CTX_IDX[1], :])
            ctxs.append((ctx0, ctx1))

        wbT_t = pool.tile([128, KC * INNER], BF16)
        wbT_c = wbT.rearrange("(p x) m -> p (x m)", p=128)
        step = KC * INNER // 3
        for j in (1, 2, 0):
            sl = slice(j * step, (j + 1) * step)
            nc.sync.dma_start(wbT_t[:, sl], wbT_c[:, sl])

        # --- ctx transposes into featT ---
        featT = pool.tile([128, KC * 256], BF16)
        featT3 = featT[:].rearrange("p (si rest) -> p si rest", si=3)
        for h in range(2):
            ctx0, ctx1 = ctxs[h]
            for c in range(HC):
                tpc = pst.tile([128, 2 * 128], F32, tag="tpc")
                for si, src in enumerate((ctx0, ctx1)):
                    nc.tensor.transpose(tpc[:, si * 128:(si + 1) * 128],
                                        src[:, c * 128:(c + 1) * 128], id_f32)
                col = c * 256 + h * 128
                nc.scalar.copy(featT3[:, 1:3, col:col + 128],
                               tpc[:].rearrange("p (si x) -> p si x", si=2))

        # --- ctx part of the base linear (during the gather) ---
        hiddenT = pool.tile([128, 2 * 256], BF16)
        accs = [[psh.tile([128, 128], F32, tag=f"acc{mt}{h}", bufs=1,
                          name=f"acc{mt}{h}") for h in range(2)]
                for mt in range(2)]
        for c in range(HC, KC):
            for mt in range(2):
                for h in range(2):
                    nc.tensor.matmul(
                        accs[mt][h][:],
                        lhsT=wbT_t[:, c * INNER + mt * 128: c * INNER + (mt + 1) * 128],
                        rhs=featT[:, c * 256 + h * 128: c * 256 + h * 128 + 128],
                        start=(c == HC), stop=False,
                    )

        # --- early center partials: X = A_L + A_R; X' = X/len + O/len ---
        xps = []
        for h in range(2):
            car = cars[h]
            osc = gpool.tile([128, H], F32, tag=f"osc{h}", bufs=1)
            nc.vector.tensor_scalar(osc[:], car[:, O], rcps[h][:, :1], None,
                                    op0=mybir.AluOpType.mult)
            x = gpool.tile([128, H], F32, tag=f"x{h}", bufs=1)
            nc.vector.tensor_tensor(out=x[:], in0=car[:, 0:H], in1=car[:, H:2 * H],
                                    op=mybir.AluOpType.add)
            xp = gpool.tile([128, H], F32, tag=f"xp{h}", bufs=1)
            nc.vector.scalar_tensor_tensor(
                out=xp[:], in0=x[:], scalar=rcps[h][:, :1], in1=osc[:],
                op0=mybir.AluOpType.mult, op1=mybir.AluOpType.add)
            xps.append(xp)

        # --- per half: finish center, transpose, close base linear, experts ---
        out3 = pool.tile([128, 2 * NB_LABELS], F32)
        for h in range(2):
            b0 = h * 128
            car = cars[h]
            y = gpool.tile([128, H], F32, tag=f"y{h}", bufs=1)
            nc.vector.tensor_tensor(out=y[:], in0=car[:, 2 * H:3 * H],
                                    in1=car[:, 3 * H:4 * H],
                                    op=mybir.AluOpType.add)
            center = gpool.tile([128, H], BF16, tag=f"center{h}", bufs=1)
            nc.vector.scalar_tensor_tensor(
                out=center[:], in0=y[:], scalar=rcps[h][:, :1], in1=xps[h][:],
                op0=mybir.AluOpType.mult, op1=mybir.AluOpType.add)

            for c in range(HC):
                tp = pst.tile([128, 128], BF16, tag="tpc")
                nc.tensor.transpose(tp[:], center[:, c * 128:(c + 1) * 128],
                                    id_bf[:])
                col = c * 256 + b0
                if c % 2 == 0:
                    nc.vector.tensor_copy(featT3[:, 0:1, col:col + 128],
                                          tp[:].rearrange("p (si x) -> p si x", si=1))
                else:
                    nc.scalar.copy(featT3[:, 0:1, col:col + 128],
                                   tp[:].rearrange("p (si x) -> p si x", si=1))

            for c in range(HC):
                for mt in range(2):
                    nc.tensor.matmul(
                        accs[mt][h][:],
                        lhsT=wbT_t[:, c * INNER + mt * 128: c * INNER + (mt + 1) * 128],
                        rhs=featT[:, c * 256 + b0: c * 256 + b0 + 128],
                        start=False, stop=(c == HC - 1),
                    )
            for mt in range(2):
                nc.scalar.activation(hiddenT[:, mt * 256 + b0: mt * 256 + b0 + 128],
                                     accs[mt][h][:],
                                     mybir.ActivationFunctionType.Relu,
                                     bias=bb_t[:, mt:mt + 1], scale=1.0)

            catf = spool.tile([128, 1], F32, tag=f"catf{h}", bufs=1)
            nc.vector.tensor_copy(catf[:], csti_t[:, 12 + h:13 + h])
            mask36 = spool.tile([128, NE], F32, tag=f"mask36{h}", bufs=1)
            nc.vector.tensor_scalar(mask36[:], io36f, catf[:, :1], None,
                                    op0=mybir.AluOpType.is_equal)
            ps36 = pst.tile([128, NE], F32, tag="tpc", name=f"ps36{h}")
            nc.tensor.matmul(ps36[:], lhsT=hiddenT[:, b0:b0 + 128],
                             rhs=wexpA[:], start=True, stop=False)
            nc.tensor.matmul(ps36[:], lhsT=hiddenT[:, 256 + b0:256 + b0 + 128],
                             rhs=wexpB[:], start=False, stop=False)
            nc.tensor.matmul(ps36[:], lhsT=ones1[:, b0:b0 + 128],
                             rhs=wexpC[:], start=False, stop=True)

            prod = spool.tile([128, NE], F32, tag=f"prod{h}", bufs=1)
            nc.vector.tensor_tensor(out=prod[:], in0=ps36[:], in1=mask36[:],
                                    op=mybir.AluOpType.mult)
            nc.vector.tensor_reduce(
                out=out3[:, h * NB_LABELS:(h + 1) * NB_LABELS],
                in_=prod[:].rearrange("p (e n) -> p n e", n=NB_LABELS),
                axis=mybir.AxisListType.X, op=mybir.AluOpType.add)
            nc.sync.dma_start(
                out.rearrange("(g p) n -> p g n", p=128)[:, h:h + 1, :],
                out3[:].rearrange("p (g n) -> p g n", n=NB_LABELS)[:, h:h + 1, :])

    nc.compile()
    return nc


_NC = None


def _get_nc():
    global _NC
    if _NC is None:
        _NC = _build()
    return _NC


def _const_blobs(b_base, W_experts, b_experts):
    cstf = np.zeros((128, C_NF), dtype=np.float32)
    cstf[:, C_ID:C_ID + 128] = np.eye(128, dtype=np.float32)
    cstf[:, C_IO36:C_IO36 + NE] = np.repeat(
        np.arange(NB_EXPERTS, dtype=np.float32), NB_LABELS)[None, :]
    bb = np.asarray(b_base, dtype=np.float32)
    cstf[:, C_BB:C_BB + 2] = bb.reshape(2, 128).T
    we = np.asarray(W_experts, dtype=np.float32)
    wexp = we.transpose(2, 0, 1).reshape(INNER, NE)
    cstf[:, C_WA:C_WA + NE] = wexp[0:128]
    cstf[:, C_WB:C_WB + NE] = wexp[128:256]
    cstf[0, C_WC:C_WC + NE] = np.asarray(b_experts, np.float32).reshape(NE)
    return cstf


def _prep_inputs(embeddings, position_indexes, categories, W_base, b_base,
                 W_experts, b_experts):
    emb = np.ascontiguousarray(np.asarray(embeddings, dtype=np.float32)).reshape(
        NCORES, BC * S, H)
    pos = np.asarray(position_indexes).astype(np.int64).reshape(NCORES, BC, 2)
    cat = np.asarray(categories).astype(np.int32).reshape(NCORES, BC)
    wb = np.asarray(W_base, dtype=np.float32)
    wbT = np.ascontiguousarray(
        wb.T.reshape(KC, 128, INNER).transpose(1, 0, 2).reshape(128, KC * INNER)
    ).reshape(F3H, INNER).astype(ml_dtypes.bfloat16)
    cstf = _const_blobs(b_base, W_experts, b_experts)

    BIG = 10**6
    starts = pos[:, :, 0]
    lens = pos[:, :, 1] - pos[:, :, 0]
    base = np.arange(BC, dtype=np.int64) * S
    i0 = base[None, :] + starts
    cols = []
    for k in range(4):
        cols.append(np.where(lens >= 2 * k + 2, i0 + 2 * k, BIG))
    cols.append(np.where(lens % 2 == 1, i0 + lens - 1, BIG))
    cols.append(lens)
    cols.append(cat.astype(np.int64))
    gi = np.stack(cols, axis=-1).reshape(NCORES, 2, 128, 7)
    csti = np.zeros((NCORES, 128, 14), np.int32)
    csti[:, :, 0:5] = gi[:, 0, :, 0:5]
    csti[:, :, 5:10] = gi[:, 1, :, 0:5]
    csti[:, :, 10] = gi[:, 0, :, 5]
    csti[:, :, 11] = gi[:, 1, :, 5]
    csti[:, :, 12] = gi[:, 0, :, 6]
    csti[:, :, 13] = gi[:, 1, :, 6]
    csti = np.ascontiguousarray(csti)

    return [
        {"emb": emb[i], "wbT": wbT, "cstf": cstf, "csti": csti[i]}
        for i in range(NCORES)
    ]


def _run(in_maps, **kw):
    nc = _get_nc()
    return run_bass_kernel_spmd(nc, in_maps, core_ids=list(range(NCORES)), **kw)


def kernel(embeddings, position_indexes, categories, W_base, b_base, W_experts,
           b_experts):
    in_maps = _prep_inputs(embeddings, position_indexes, categories, W_base,
                           b_base, W_experts, b_experts)
    res = _run(in_maps)
    return np.concatenate([r["out"] for r in res.results], axis=0)
